# revision 44
# baseline (speedup 1.0000x reference)
"""Trainium2 Bass kernel for nn_CapsuleSequenceToGraph.

Strategy (8 NeuronCores, single SPMD NEFF):
  - Shard the sequence dim T across cores (weights are the dominant HBM
    traffic; T-sharding reads each weight byte exactly once chip-wide).
  - Inputs x and W are pre-converted to bf16 on the host and packed 4 t-pairs
    per DRAM row-group: [128, 4*640] = 4x [block-diag x (128) | W (512)].
    pri = einsum('btj,tnjd->btnd') via one bf16 PE matmul per t-pair,
    pri kept in SBUF as bf16, tile layout [part=(t2,b=64), free=(d,n)].
  - Dynamic routing (3 rounds + final readout):
      s_r = sum_t softmax_n(b_r) * pri   -> cross-core AllReduce of [B, n*d]
      v_r = tanh(s_r);  V_r = sum v_r    (running sum)
      b_{r+1} = sum_d V_r * pri          (fresh each round; b_0 = 0)
    Round 0's softmax over zeros is uniform, so s_0 = sum_t pri / 32 is
    accumulated directly on the PE while pri is being produced.
    The t/tile reduction of s runs on the PE via a stacked-identity selector
    with PSUM accumulation.  Elementwise muls run on DVE with most w-muls
    offloaded to Pool (gpsimd); d-contraction reduces on DVE (only engine
    with grouped free-dim reduce).  exp/tanh and PSUM->SBUF copies on Act.
    |b| < 0.02 for these inputs so softmax needs no max-subtraction.
  - All 4 modalities' s partials are packed into ONE bf16 AllReduce of
    [256, FN] per routing round (3 collectives total instead of 12): on HW
    the per-collective fixed cost dominates, so minimizing collective count
    beats overlapping smaller ones.  The collective is emitted one round
    late on the gpsimd queue so its seq-wait never stalls Pool muls.
  - The final s_3 is NOT allreduced: each core emits its partial sum and the
    host reduces + applies tanh (saves one collective round per modality).
"""

import sys

if "/opt/trn_rl_repo" not in sys.path:
    sys.path.insert(0, "/opt/trn_rl_repo")

import numpy as np
import ml_dtypes

import concourse.bass as bass
import concourse.bacc as bacc
import concourse.mybir as mybir
from concourse import tile
from concourse import library_config
from concourse.bass_utils import run_bass_kernel_spmd

F32 = mybir.dt.float32
BF16 = mybir.dt.bfloat16
AF = mybir.ActivationFunctionType
ALU = mybir.AluOpType

N_CORES = 8
B = 64
NV = 32  # n vertices
DC = 16  # capsule dim
J = 64  # MULT_D
T_DIMS = {"text": 128, "audio": 512, "video": 256, "frames": 256}
W_NAMES = {"text": "W_tpc", "audio": "W_apc", "video": "W_vpc", "frames": "W_fpc"}
# Collective chains. One chain = one batched AllReduce per routing round
# (3 total): on HW the per-collective fixed cost dominates any lost
# compute/collective overlap, so fewer, bigger AllReduces win.
CHAINS = [["text", "video", "frames", "audio"]]
P1_ORDER = ["text", "video", "frames", "audio"]
OUT_ORDER = ["text", "audio", "video", "frames"]
ORDER = OUT_ORDER  # modality enumeration (test.py compat)
ROUNDS = 3
FN = DC * NV  # 512, free dim (d-major: flat = d*32 + n)
PG = 4  # t-pairs per load DMA
AR_MODE = "cc"  # "rdma": peer SBUF broadcasts; "cc": collective_compute

_CACHE = {}


def _pairs(mod):
    return T_DIMS[mod] // N_CORES // 2


def _build():
    nc = bacc.Bacc("TRN2", target_bir_lowering=False, debug=False, num_devices=N_CORES)
    if AR_MODE == "rdma":
        # the sim race detector's RDMA watermark discipline doesn't fit the
        # butterfly (distinct buffer per step, shared sem); sim-only knob
        nc.detect_race_conditions = False

    xw_d = {}
    out_d = {}
    for mod in T_DIMS:
        P = _pairs(mod)
        G = max(1, P // PG)
        xw_d[mod] = nc.dram_tensor(f"xw_{mod}", [G, 128, (P // G) * 640], BF16,
                                   kind="ExternalInput")
        out_d[mod] = nc.dram_tensor(f"out_{mod}", [B, FN], F32, kind="ExternalOutput")
    sel_d = nc.dram_tensor("sel", [128, 64], BF16, kind="ExternalInput")
    # runtime wait thresholds for the butterfly allreduce; zeros during the
    # tile scheduling pass (no inputs) so remote-sem waits don't deadlock it
    bthr_d = nc.dram_tensor("bthr", [1, 32], mybir.dt.int32, kind="ExternalInput")

    rg = [list(range(N_CORES))]

    with tile.TileContext(nc) as tc:
        with (
            tc.tile_pool(name="io", bufs=3) as io,
            tc.tile_pool(name="pri", bufs=1) as pri_pool,
            tc.tile_pool(name="state", bufs=1) as st,
            tc.tile_pool(name="wk", bufs=8) as wk,
            tc.tile_pool(name="sm", bufs=2) as sm,
            tc.tile_pool(name="pp", bufs=4, space="PSUM") as ps_pri,
            tc.tile_pool(name="psacc", bufs=1, space="PSUM") as ps_s,
            tc.tile_pool(name="dram", bufs=1, space="DRAM") as dr,
        ):
            sel = st.tile([128, 64], BF16, tag="sel", name="sel")
            nc.sync.dma_start(sel[:], sel_d[:])

            if AR_MODE == "rdma":
                # one gpsimd ucode library covering tensor_tensor AND the
                # remote-DMA desc-gen ops: avoids per-switch library reloads
                nc.gpsimd.load_library(library_config.proxy)
                rsem = [nc.alloc_semaphore(f"ars{ci}") for ci in range(2)]
                lsem = nc.alloc_semaphore("arl")
                for s_ in rsem + [lsem]:
                    nc.gpsimd.sem_clear(s_)
                rthr = [0, 0]  # cumulative remote-sem thresholds per chain
                xstate = {}  # (ci) -> current butterfly partial tile
                # Scheduling-pass escape: the no-exec tile scheduler runs with
                # zeroed inputs and has no peers, so remote-sem waits would
                # deadlock it.  A runtime flag (bthr[31], 0 in scheduling / 1
                # at runtime) gates a pre-credit of every remote-facing sem.
                flagreg = nc.gpsimd.alloc_register("rt_flag")
                nc.gpsimd.reg_load(flagreg, bthr_d[0:1, 31:32])
                flagval = nc.gpsimd.snap(flagreg, donate=True, min_val=0, max_val=1)
                with tc.If(flagval == 0):
                    nc.gpsimd.sem_inc(rsem[0], 999)
                    nc.gpsimd.sem_inc(rsem[1], 999)
                    nc.gpsimd.sem_inc(nc._bir_kernel_barrier_sem, 99)
                # register the entry-barrier AllGather; waits are attached to
                # the first remote prep of each chain (standalone wait_ge
                # instructions get hoisted/merged by the tile scheduler)
                nc._bir_kernel_barrier_sem_replica_groups.extend(
                    set(g) for g in rg
                )
                # butterfly peer Δtpb per step; cross-die (bit 2) dest must sit
                # in a D2D-capable slot (4-7)
                BFLY_DESTS = [
                    [(0, 1)] + [None] * 7,
                    [(0, 2)] + [None] * 7,
                    [None] * 4 + [(0, 4)] + [None] * 3,
                ]

            pri = {}  # mod -> list of [128, FN] bf16 tiles
            vvbf = {}
            Vf = {}
            bstate = {}
            estate = {}
            den = {}
            rinv = {}
            rcbf = {}
            s_glob = {}
            s_ps = {}
            arbufs = {}  # (ci, r) -> (bi, bo) DRAM tiles

            def alloc_state(mod):
                P = _pairs(mod)
                vvbf[mod] = st.tile([128, FN], BF16, tag=f"vv_{mod}", name=f"vv_{mod}")
                Vf[mod] = st.tile([64, FN], F32, tag=f"V_{mod}", name=f"V_{mod}")
                bstate[mod] = st.tile([128, P * NV], F32, tag=f"b_{mod}", name=f"b_{mod}")
                estate[mod] = st.tile([128, P * NV], BF16, tag=f"e_{mod}", name=f"e_{mod}")
                den[mod] = st.tile([128, P], F32, tag=f"den_{mod}", name=f"den_{mod}")
                rinv[mod] = st.tile([128, P], F32, tag=f"ri_{mod}", name=f"ri_{mod}")
                rcbf[mod] = st.tile([128, P * NV], BF16, tag=f"rc_{mod}", name=f"rc_{mod}")

            def bfly_step(ci, r, k, x):
                """One recursive-doubling step: send x to peer (me XOR 2^k)'s
                buf, wait for the peer's x, return x + peer_x."""
                buf = st.tile([128, FN], BF16, tag=f"bf_{ci}_{r}_{k}",
                              name=f"bf_{ci}_{r}_{k}")
                prep = nc.gpsimd.remote_dma_broadcast(
                    buf[:], x[:],
                    remote_sem=rsem[ci], local_sem=lsem,
                    rdests=BFLY_DESTS[k],
                )
                if r == 0 and k == 0:
                    prep._wait_ge(nc._bir_kernel_barrier_sem, 1)
                nc.gpsimd.trigger_dma(count=None)
                rthr[ci] += 2
                nx = st.tile([128, FN], BF16, tag=f"bx_{ci}_{r}_{k}",
                             name=f"bx_{ci}_{r}_{k}")
                add = nc.vector.tensor_tensor(
                    out=nx[:], in0=x[:], in1=buf[:], op=ALU.add
                )
                add._wait_ge(rsem[ci], rthr[ci])
                return nx

            def emit_send_rdma(ci, r):
                """Pack both chain mods' PSUM s partials into a [128, FN] bf16
                SBUF tile and fire butterfly step 0."""
                snd = st.tile([128, FN], BF16, tag=f"snd_{ci}_{r}",
                              name=f"snd_{ci}_{r}")
                for mi, mod in enumerate(CHAINS[ci]):
                    nc.scalar.copy(snd[mi * 64 : (mi + 1) * 64, :], s_ps[mod][:])
                xstate[ci] = bfly_step(ci, r, 0, snd)

            def land_rdma(ci, r):
                """Finish butterfly steps 1..2; set per-mod s views."""
                x = xstate[ci]
                for k in (1, 2):
                    x = bfly_step(ci, r, k, x)
                for mi, mod in enumerate(CHAINS[ci]):
                    s_glob[mod] = x[mi * 64 : (mi + 1) * 64, :]

            def emit_send(ci, r):
                """Copy both chain mods' PSUM s partials into the chain's
                DRAM collective input buffer."""
                if AR_MODE == "rdma":
                    emit_send_rdma(ci, r)
                    return
                mods = CHAINS[ci]
                rows = 64 * len(mods)
                bi = dr.tile([rows, FN], BF16, tag=f"ari_{ci}_{r}", name=f"ari_{ci}_{r}")
                bo = dr.tile([rows, FN], BF16, tag=f"aro_{ci}_{r}", name=f"aro_{ci}_{r}")
                arbufs[(ci, r)] = (bi, bo)
                for mi, mod in enumerate(mods):
                    s_loc = sm.tile([64, FN], BF16, tag=f"sl_{mod}", name=f"sl_{mod}")
                    nc.scalar.copy(s_loc[:], s_ps[mod][:])
                    nc.sync.dma_start(bi[mi * 64 : (mi + 1) * 64, :], s_loc[:])

            def emit_collective(ci, r):
                """One bf16 AllReduce per chain-round (gpsimd queue). Emitted
                a half-round late so its seq-wait never stalls Pool muls."""
                if AR_MODE == "rdma":
                    return  # send+trigger already emitted in emit_send_rdma
                bi, bo = arbufs[(ci, r)]
                nc.gpsimd.collective_compute(
                    "AllReduce",
                    ALU.add,
                    replica_groups=rg,
                    ins=[bi.opt()],
                    outs=[bo.opt()],
                )

            def emit_allreduce(ci, r):
                emit_send(ci, r)
                emit_collective(ci, r)

            def land_allreduce(ci, r):
                """Land the chain's allreduced s into per-mod SBUF views."""
                if AR_MODE == "rdma":
                    land_rdma(ci, r)
                    return
                bo = arbufs[(ci, r)][1]
                for mi, mod in enumerate(CHAINS[ci]):
                    sg = sm.tile([64, FN], BF16, tag=f"sg_{mod}", name=f"sg_{mod}")
                    nc.sync.dma_start(sg[:], bo[mi * 64 : (mi + 1) * 64, :])
                    s_glob[mod] = sg

            # ---------- phase 1: pri + s0 accumulation ----------
            ncopy = [0]

            def phase1(mod):
                P = _pairs(mod)
                G = max(1, P // PG)
                gp = P // G
                alloc_state(mod)
                pri[mod] = []
                s_ps[mod] = ps_s.tile([64, FN], F32, tag=f"s_{mod}", name=f"s_{mod}")
                for g in range(G):
                    xw_t = io.tile([128, gp * 640], BF16, tag=f"xw{gp}", name="xw_t")
                    nc.sync.dma_start(xw_t[:], xw_d[mod][g])
                    for k in range(gp):
                        p = g * gp + k
                        pp = ps_pri.tile([128, FN], F32, tag="pp", name="pp")
                        nc.tensor.matmul(
                            pp[:],
                            xw_t[:, k * 640 : k * 640 + 128],
                            xw_t[:, k * 640 + 128 : (k + 1) * 640],
                            start=True, stop=True,
                        )
                        pri_t = pri_pool.tile([128, FN], BF16, tag=f"pri_{mod}_{p}",
                                              name=f"pri_{mod}_{p}")
                        # PSUM->SBUF bf16 downcast: alternate Act / DVE
                        if ncopy[0] % 2 == 0:
                            nc.scalar.copy(pri_t[:], pp[:])
                        else:
                            nc.vector.tensor_copy(pri_t[:], pp[:])
                        ncopy[0] += 1
                        pri[mod].append(pri_t)
                        nc.tensor.matmul(
                            s_ps[mod][:], sel[:], pri_t[:],
                            start=(p == 0), stop=(p == P - 1),
                        )

            for mod in P1_ORDER:
                phase1(mod)
                for ci, mods in enumerate(CHAINS):
                    if mod == mods[-1]:
                        emit_allreduce(ci, 0)

            # ---------- phase 2: routing rounds ----------
            def v_update_and_b(mod, r):
                """tanh(s_r) -> V; w = pri*V; b = sum_d w; softmax prep."""
                P = _pairs(mod)
                t_tmp = sm.tile([64, FN], F32, tag=f"vt_{mod}", name=f"vt_{mod}")
                scale = (1.0 / NV) if r == 0 else 1.0
                nc.scalar.activation(t_tmp[:], s_glob[mod][:], AF.Tanh, scale=scale)
                if r == 0:
                    nc.vector.tensor_copy(Vf[mod][:], t_tmp[:])
                else:
                    nc.gpsimd.tensor_tensor(
                        out=Vf[mod][:], in0=Vf[mod][:], in1=t_tmp[:], op=ALU.add
                    )
                nc.scalar.copy(vvbf[mod][0:64, :], Vf[mod][:])
                nc.scalar.copy(vvbf[mod][64:128, :], Vf[mod][:])
                for p in range(P):
                    w_t = wk.tile([128, FN], BF16, tag="w", name="w_t")
                    # w-mult on Pool (gpsimd) so DVE keeps the reduces
                    nc.gpsimd.tensor_tensor(
                        out=w_t[:], in0=pri[mod][p][:], in1=vvbf[mod][:], op=ALU.mult
                    )
                    nc.vector.tensor_reduce(
                        out=bstate[mod][:, p * NV : (p + 1) * NV],
                        in_=w_t.rearrange("q (d n) -> q n d", d=DC),
                        axis=mybir.AxisListType.X,
                        op=ALU.add,
                    )
                nc.scalar.activation(estate[mod][:], bstate[mod][:], AF.Exp)
                nc.vector.tensor_reduce(
                    out=den[mod][:],
                    in_=estate[mod].rearrange("q (t n) -> q t n", n=NV),
                    axis=mybir.AxisListType.X,
                    op=ALU.add,
                )
                nc.vector.reciprocal(rinv[mod][:], den[mod][:])
                nc.gpsimd.tensor_tensor(
                    out=rcbf[mod].rearrange("q (t n) -> q t n", n=NV),
                    in0=estate[mod].rearrange("q (t n) -> q t n", n=NV),
                    in1=rinv[mod].unsqueeze(2).broadcast_to([128, P, NV]),
                    op=ALU.mult,
                )

            def mul1_and_s(mod):
                """m = rc * pri ; s_psum = sum_t m via selector matmuls."""
                P = _pairs(mod)
                for p in range(P):
                    m_t = wk.tile([128, FN], BF16, tag="m", name="m_t")
                    # m-mult: 5/6 on Pool, 1/6 on DVE (load balance)
                    eng = nc.vector if (p % 6 == 0) else nc.gpsimd
                    eng.tensor_tensor(
                        out=m_t.rearrange("q (d n) -> q d n", d=DC),
                        in0=pri[mod][p].rearrange("q (d n) -> q d n", d=DC),
                        in1=rcbf[mod][:, p * NV : (p + 1) * NV]
                        .unsqueeze(1)
                        .broadcast_to([128, DC, NV]),
                        op=ALU.mult,
                    )
                    nc.tensor.matmul(
                        s_ps[mod][:], sel[:], m_t[:],
                        start=(p == 0), stop=(p == P - 1),
                    )

            pending = []
            for r in range(ROUNDS):
                for ci, mods in enumerate(CHAINS):
                    while pending:
                        emit_collective(*pending.pop(0))
                    land_allreduce(ci, r)
                    for mod in mods:
                        v_update_and_b(mod, r)
                        mul1_and_s(mod)
                    if r < ROUNDS - 1:
                        emit_send(ci, r + 1)
                        pending.append((ci, r + 1))
                    else:
                        for mod in mods:
                            s_out = sm.tile([64, FN], F32, tag=f"so_{mod}",
                                            name=f"so_{mod}")
                            nc.scalar.copy(s_out[:], s_ps[mod][:])
                            nc.sync.dma_start(out_d[mod][:], s_out[:])

    nc.compile()
    return nc


def _host_prep(inputs):
    """Build the 8 per-core input maps (T-sharded, PE-ready bf16 layouts)."""
    sel = np.concatenate([np.eye(64, dtype=np.float32)] * 2, axis=0).astype(
        ml_dtypes.bfloat16
    )
    bthr = np.zeros((1, 32), np.int32)
    bthr[0, 31] = 1  # runtime flag: disables the scheduling-pass sem pre-credit
    in_maps = []
    for c in range(N_CORES):
        m = {"sel": sel, "bthr": bthr}
        for mod in T_DIMS:
            T = T_DIMS[mod]
            Tc = T // N_CORES
            P = Tc // 2
            G = max(1, P // PG)
            gp = P // G
            t0 = c * Tc
            x = np.asarray(inputs[mod], dtype=np.float32)  # [B, T, J]
            W = np.asarray(inputs[W_NAMES[mod]], dtype=np.float32)  # [T,NV,J,DC]
            xs = np.ascontiguousarray(
                x[:, t0 : t0 + Tc, :].transpose(1, 2, 0)
            )  # [Tc, J, B]
            wt = W[t0 : t0 + Tc].transpose(0, 2, 3, 1).reshape(Tc, J, FN)
            # wt[t, j, d*32+n] = W[t, n, j, d]
            xw = np.zeros((P, 128, 640), dtype=ml_dtypes.bfloat16)
            xw[:, 0:64, 0:64] = xs[0::2]
            xw[:, 64:128, 64:128] = xs[1::2]
            xw[:, 0:64, 128:] = wt[0::2]
            xw[:, 64:128, 128:] = wt[1::2]
            # pack gp pairs per row-group: [G, 128, gp*640]
            m[f"xw_{mod}"] = np.ascontiguousarray(
                xw.reshape(G, gp, 128, 640).transpose(0, 2, 1, 3).reshape(
                    G, 128, gp * 640
                )
            )
        in_maps.append(m)
    return in_maps


def _gather(results):
    outs = []
    for mod in OUT_ORDER:
        s = np.zeros((B, FN), dtype=np.float64)
        for c in range(N_CORES):
            s += np.asarray(results[c][f"out_{mod}"], dtype=np.float64)
        o = np.tanh(s.astype(np.float32))
        outs.append(np.ascontiguousarray(o.reshape(B, DC, NV).transpose(0, 2, 1)))
    return tuple(outs)


def kernel(**inputs):
    if "nc" not in _CACHE:
        _CACHE["nc"] = _build()
    nc = _CACHE["nc"]
    in_maps = _host_prep(inputs)
    res = run_bass_kernel_spmd(nc, in_maps, core_ids=list(range(N_CORES)))
    return _gather(res.results)


# revision 45
# speedup vs baseline: 2.7829x; 2.7829x over previous
"""Trainium2 Bass kernel for nn_CapsuleSequenceToGraph.

Strategy (8 NeuronCores, single SPMD NEFF):
  - Shard the sequence dim T across cores (weights are the dominant HBM
    traffic; T-sharding reads each weight byte exactly once chip-wide).
  - Inputs x and W are pre-converted to bf16 on the host and packed 4 t-pairs
    per DRAM row-group: [128, 4*640] = 4x [block-diag x (128) | W (512)].
    pri = einsum('btj,tnjd->btnd') via one bf16 PE matmul per t-pair,
    pri kept in SBUF as bf16, tile layout [part=(t2,b=64), free=(d,n)].
  - Dynamic routing (3 rounds + final readout):
      s_r = sum_t softmax_n(b_r) * pri   -> cross-core AllReduce of [B, n*d]
      v_r = tanh(s_r);  V_r = sum v_r    (running sum)
      b_{r+1} = sum_d V_r * pri          (fresh each round; b_0 = 0)
    Round 0's softmax over zeros is uniform, so s_0 = sum_t pri / 32 is
    accumulated directly on the PE while pri is being produced.
    The t/tile reduction of s runs on the PE via a stacked-identity selector
    with PSUM accumulation.  Elementwise muls run on DVE with most w-muls
    offloaded to Pool (gpsimd); d-contraction reduces on DVE (only engine
    with grouped free-dim reduce).  exp/tanh and PSUM->SBUF copies on Act.
    |b| < 0.02 for these inputs so softmax needs no max-subtraction.
  - All 4 modalities' s partials are packed into ONE bf16 AllReduce of
    [256, FN] per routing round (3 collectives total instead of 12): on HW
    the per-collective fixed cost dominates, so minimizing collective count
    beats overlapping smaller ones.  The collective is emitted one round
    late on the gpsimd queue so its seq-wait never stalls Pool muls.
  - The final s_3 is NOT allreduced: each core emits its partial sum and the
    host reduces + applies tanh (saves one collective round per modality).
"""

import sys

if "/opt/trn_rl_repo" not in sys.path:
    sys.path.insert(0, "/opt/trn_rl_repo")

import numpy as np
import ml_dtypes

import concourse.bass as bass
import concourse.bacc as bacc
import concourse.mybir as mybir
from concourse import tile
from concourse import library_config
from concourse.bass_utils import run_bass_kernel_spmd

F32 = mybir.dt.float32
BF16 = mybir.dt.bfloat16
AF = mybir.ActivationFunctionType
ALU = mybir.AluOpType

N_CORES = 8
B = 64
NV = 32  # n vertices
DC = 16  # capsule dim
J = 64  # MULT_D
T_DIMS = {"text": 128, "audio": 512, "video": 256, "frames": 256}
W_NAMES = {"text": "W_tpc", "audio": "W_apc", "video": "W_vpc", "frames": "W_fpc"}
# Collective chains. One chain = one batched AllReduce per routing round
# (3 total): on HW the per-collective fixed cost dominates any lost
# compute/collective overlap, so fewer, bigger AllReduces win.
CHAINS = [["text", "video", "frames", "audio"]]
P1_ORDER = ["text", "video", "frames", "audio"]
OUT_ORDER = ["text", "audio", "video", "frames"]
ORDER = OUT_ORDER  # modality enumeration (test.py compat)
ROUNDS = 3
FN = DC * NV  # 512, free dim (d-major: flat = d*32 + n)
PG = 4  # t-pairs per load DMA
AR_MODE = "cc"  # "rdma": peer SBUF broadcasts; "cc": collective_compute

_CACHE = {}


def _pairs(mod):
    return T_DIMS[mod] // N_CORES // 2


def _build():
    nc = bacc.Bacc("TRN2", target_bir_lowering=False, debug=False, num_devices=N_CORES)
    if AR_MODE == "rdma":
        # the sim race detector's RDMA watermark discipline doesn't fit the
        # butterfly (distinct buffer per step, shared sem); sim-only knob
        nc.detect_race_conditions = False

    xw_d = {}
    out_d = {}
    for mod in T_DIMS:
        P = _pairs(mod)
        G = max(1, P // PG)
        xw_d[mod] = nc.dram_tensor(f"xw_{mod}", [G, 128, (P // G) * 640], BF16,
                                   kind="ExternalInput")
        out_d[mod] = nc.dram_tensor(f"out_{mod}", [B, FN], F32, kind="ExternalOutput")
    sel_d = nc.dram_tensor("sel", [128, 64], BF16, kind="ExternalInput")
    # runtime wait thresholds for the butterfly allreduce; zeros during the
    # tile scheduling pass (no inputs) so remote-sem waits don't deadlock it
    bthr_d = nc.dram_tensor("bthr", [1, 32], mybir.dt.int32, kind="ExternalInput")

    rg = [list(range(N_CORES))]

    with tile.TileContext(nc) as tc:
        with (
            tc.tile_pool(name="io", bufs=3) as io,
            tc.tile_pool(name="pri", bufs=1) as pri_pool,
            tc.tile_pool(name="state", bufs=1) as st,
            tc.tile_pool(name="wk", bufs=8) as wk,
            tc.tile_pool(name="sm", bufs=2) as sm,
            tc.tile_pool(name="pp", bufs=4, space="PSUM") as ps_pri,
            tc.tile_pool(name="psacc", bufs=1, space="PSUM") as ps_s,
            tc.tile_pool(name="dram", bufs=1, space="DRAM") as dr,
        ):
            sel = st.tile([128, 64], BF16, tag="sel", name="sel")
            nc.sync.dma_start(sel[:], sel_d[:])

            if AR_MODE == "rdma":
                # one gpsimd ucode library covering tensor_tensor AND the
                # remote-DMA desc-gen ops: avoids per-switch library reloads
                nc.gpsimd.load_library(library_config.proxy)
                rsem = [nc.alloc_semaphore(f"ars{ci}") for ci in range(2)]
                lsem = nc.alloc_semaphore("arl")
                for s_ in rsem + [lsem]:
                    nc.gpsimd.sem_clear(s_)
                rthr = [0, 0]  # cumulative remote-sem thresholds per chain
                xstate = {}  # (ci) -> current butterfly partial tile
                # Scheduling-pass escape: the no-exec tile scheduler runs with
                # zeroed inputs and has no peers, so remote-sem waits would
                # deadlock it.  A runtime flag (bthr[31], 0 in scheduling / 1
                # at runtime) gates a pre-credit of every remote-facing sem.
                flagreg = nc.gpsimd.alloc_register("rt_flag")
                nc.gpsimd.reg_load(flagreg, bthr_d[0:1, 31:32])
                flagval = nc.gpsimd.snap(flagreg, donate=True, min_val=0, max_val=1)
                with tc.If(flagval == 0):
                    nc.gpsimd.sem_inc(rsem[0], 999)
                    nc.gpsimd.sem_inc(rsem[1], 999)
                    nc.gpsimd.sem_inc(nc._bir_kernel_barrier_sem, 99)
                # register the entry-barrier AllGather; waits are attached to
                # the first remote prep of each chain (standalone wait_ge
                # instructions get hoisted/merged by the tile scheduler)
                nc._bir_kernel_barrier_sem_replica_groups.extend(
                    set(g) for g in rg
                )
                # butterfly peer Δtpb per step; cross-die (bit 2) dest must sit
                # in a D2D-capable slot (4-7)
                BFLY_DESTS = [
                    [(0, 1)] + [None] * 7,
                    [(0, 2)] + [None] * 7,
                    [None] * 4 + [(0, 4)] + [None] * 3,
                ]

            pri = {}  # mod -> list of [128, FN] bf16 tiles
            vvbf = {}
            Vf = {}
            bstate = {}
            estate = {}
            den = {}
            rinv = {}
            rcbf = {}
            s_glob = {}
            s_ps = {}
            arbufs = {}  # (ci, r) -> (bi, bo) DRAM tiles

            def alloc_state(mod):
                P = _pairs(mod)
                vvbf[mod] = st.tile([128, FN], BF16, tag=f"vv_{mod}", name=f"vv_{mod}")
                Vf[mod] = st.tile([64, FN], F32, tag=f"V_{mod}", name=f"V_{mod}")
                bstate[mod] = st.tile([128, P * NV], F32, tag=f"b_{mod}", name=f"b_{mod}")
                estate[mod] = st.tile([128, P * NV], BF16, tag=f"e_{mod}", name=f"e_{mod}")
                den[mod] = st.tile([128, P], F32, tag=f"den_{mod}", name=f"den_{mod}")
                rinv[mod] = st.tile([128, P], F32, tag=f"ri_{mod}", name=f"ri_{mod}")
                rcbf[mod] = st.tile([128, P * NV], BF16, tag=f"rc_{mod}", name=f"rc_{mod}")

            def bfly_step(ci, r, k, x):
                """One recursive-doubling step: send x to peer (me XOR 2^k)'s
                buf, wait for the peer's x, return x + peer_x."""
                buf = st.tile([128, FN], BF16, tag=f"bf_{ci}_{r}_{k}",
                              name=f"bf_{ci}_{r}_{k}")
                prep = nc.gpsimd.remote_dma_broadcast(
                    buf[:], x[:],
                    remote_sem=rsem[ci], local_sem=lsem,
                    rdests=BFLY_DESTS[k],
                )
                if r == 0 and k == 0:
                    prep._wait_ge(nc._bir_kernel_barrier_sem, 1)
                nc.gpsimd.trigger_dma(count=None)
                rthr[ci] += 2
                nx = st.tile([128, FN], BF16, tag=f"bx_{ci}_{r}_{k}",
                             name=f"bx_{ci}_{r}_{k}")
                add = nc.vector.tensor_tensor(
                    out=nx[:], in0=x[:], in1=buf[:], op=ALU.add
                )
                add._wait_ge(rsem[ci], rthr[ci])
                return nx

            def emit_send_rdma(ci, r):
                """Pack both chain mods' PSUM s partials into a [128, FN] bf16
                SBUF tile and fire butterfly step 0."""
                snd = st.tile([128, FN], BF16, tag=f"snd_{ci}_{r}",
                              name=f"snd_{ci}_{r}")
                for mi, mod in enumerate(CHAINS[ci]):
                    nc.scalar.copy(snd[mi * 64 : (mi + 1) * 64, :], s_ps[mod][:])
                xstate[ci] = bfly_step(ci, r, 0, snd)

            def land_rdma(ci, r):
                """Finish butterfly steps 1..2; set per-mod s views."""
                x = xstate[ci]
                for k in (1, 2):
                    x = bfly_step(ci, r, k, x)
                for mi, mod in enumerate(CHAINS[ci]):
                    s_glob[mod] = x[mi * 64 : (mi + 1) * 64, :]

            def emit_send(ci, r):
                """Copy both chain mods' PSUM s partials into the chain's
                DRAM collective input buffer."""
                if AR_MODE == "rdma":
                    emit_send_rdma(ci, r)
                    return
                mods = CHAINS[ci]
                rows = 64 * len(mods)
                bi = dr.tile([rows, FN], BF16, tag=f"ari_{ci}_{r}", name=f"ari_{ci}_{r}")
                bo = dr.tile([rows, FN], BF16, tag=f"aro_{ci}_{r}", name=f"aro_{ci}_{r}")
                arbufs[(ci, r)] = (bi, bo)
                for mi, mod in enumerate(mods):
                    s_loc = sm.tile([64, FN], BF16, tag=f"sl_{mod}", name=f"sl_{mod}")
                    nc.scalar.copy(s_loc[:], s_ps[mod][:])
                    nc.sync.dma_start(bi[mi * 64 : (mi + 1) * 64, :], s_loc[:])

            def emit_collective(ci, r):
                """One bf16 AllReduce per chain-round (gpsimd queue). Emitted
                a half-round late so its seq-wait never stalls Pool muls."""
                if AR_MODE == "rdma":
                    return  # send+trigger already emitted in emit_send_rdma
                bi, bo = arbufs[(ci, r)]
                nc.gpsimd.collective_compute(
                    "AllReduce",
                    ALU.add,
                    replica_groups=rg,
                    ins=[bi.opt()],
                    outs=[bo.opt()],
                )

            def emit_allreduce(ci, r):
                emit_send(ci, r)
                emit_collective(ci, r)

            def land_allreduce(ci, r):
                """Land the chain's allreduced s into per-mod SBUF views."""
                if AR_MODE == "rdma":
                    land_rdma(ci, r)
                    return
                bo = arbufs[(ci, r)][1]
                for mi, mod in enumerate(CHAINS[ci]):
                    sg = sm.tile([64, FN], BF16, tag=f"sg_{mod}", name=f"sg_{mod}")
                    nc.sync.dma_start(sg[:], bo[mi * 64 : (mi + 1) * 64, :])
                    s_glob[mod] = sg

            # ---------- phase 1: pri + s0 accumulation ----------
            ncopy = [0]

            def phase1(mod):
                P = _pairs(mod)
                G = max(1, P // PG)
                gp = P // G
                alloc_state(mod)
                pri[mod] = []
                s_ps[mod] = ps_s.tile([64, FN], F32, tag=f"s_{mod}", name=f"s_{mod}")
                for g in range(G):
                    xw_t = io.tile([128, gp * 640], BF16, tag=f"xw{gp}", name="xw_t")
                    nc.sync.dma_start(xw_t[:], xw_d[mod][g])
                    for k in range(gp):
                        p = g * gp + k
                        pp = ps_pri.tile([128, FN], F32, tag="pp", name="pp")
                        nc.tensor.matmul(
                            pp[:],
                            xw_t[:, k * 640 : k * 640 + 128],
                            xw_t[:, k * 640 + 128 : (k + 1) * 640],
                            start=True, stop=True,
                        )
                        pri_t = pri_pool.tile([128, FN], BF16, tag=f"pri_{mod}_{p}",
                                              name=f"pri_{mod}_{p}")
                        # PSUM->SBUF bf16 downcast: alternate Act / DVE
                        if ncopy[0] % 2 == 0:
                            nc.scalar.copy(pri_t[:], pp[:])
                        else:
                            nc.vector.tensor_copy(pri_t[:], pp[:])
                        ncopy[0] += 1
                        pri[mod].append(pri_t)
                        nc.tensor.matmul(
                            s_ps[mod][:], sel[:], pri_t[:],
                            start=(p == 0), stop=(p == P - 1),
                        )

            for mod in P1_ORDER:
                phase1(mod)
                for ci, mods in enumerate(CHAINS):
                    if mod == mods[-1]:
                        emit_allreduce(ci, 0)

            # ---------- phase 2: routing rounds ----------
            def v_update_and_b(mod, r):
                """tanh(s_r) -> V; then per GROUP of pairs: w = pri*V,
                b = sum_d w, softmax, m = rc*pri, s accumulation — grouping
                lets group g's m-mults overlap group g+1's reduces."""
                P = _pairs(mod)
                GS = 4  # pairs per softmax group
                t_tmp = sm.tile([64, FN], F32, tag=f"vt_{mod}", name=f"vt_{mod}")
                scale = (1.0 / NV) if r == 0 else 1.0
                nc.scalar.activation(t_tmp[:], s_glob[mod][:], AF.Tanh, scale=scale)
                if r == 0:
                    nc.vector.tensor_copy(Vf[mod][:], t_tmp[:])
                else:
                    nc.gpsimd.tensor_tensor(
                        out=Vf[mod][:], in0=Vf[mod][:], in1=t_tmp[:], op=ALU.add
                    )
                nc.scalar.copy(vvbf[mod][0:64, :], Vf[mod][:])
                nc.scalar.copy(vvbf[mod][64:128, :], Vf[mod][:])
                for g0 in range(0, P, GS):
                    g1 = min(g0 + GS, P)
                    for p in range(g0, g1):
                        w_t = wk.tile([128, FN], BF16, tag="w", name="w_t")
                        # w-mult on Pool (gpsimd) so DVE keeps the reduces
                        nc.gpsimd.tensor_tensor(
                            out=w_t[:], in0=pri[mod][p][:], in1=vvbf[mod][:],
                            op=ALU.mult,
                        )
                        nc.vector.tensor_reduce(
                            out=bstate[mod][:, p * NV : (p + 1) * NV],
                            in_=w_t.rearrange("q (d n) -> q n d", d=DC),
                            axis=mybir.AxisListType.X,
                            op=ALU.add,
                        )
                    gn = g1 - g0
                    nc.scalar.activation(
                        estate[mod][:, g0 * NV : g1 * NV],
                        bstate[mod][:, g0 * NV : g1 * NV], AF.Exp,
                    )
                    nc.vector.tensor_reduce(
                        out=den[mod][:, g0:g1],
                        in_=estate[mod][:, g0 * NV : g1 * NV]
                        .rearrange("q (t n) -> q t n", n=NV),
                        axis=mybir.AxisListType.X,
                        op=ALU.add,
                    )
                    nc.vector.reciprocal(rinv[mod][:, g0:g1], den[mod][:, g0:g1])
                    nc.gpsimd.tensor_tensor(
                        out=rcbf[mod][:, g0 * NV : g1 * NV]
                        .rearrange("q (t n) -> q t n", n=NV),
                        in0=estate[mod][:, g0 * NV : g1 * NV]
                        .rearrange("q (t n) -> q t n", n=NV),
                        in1=rinv[mod][:, g0:g1].unsqueeze(2)
                        .broadcast_to([128, gn, NV]),
                        op=ALU.mult,
                    )
                    for p in range(g0, g1):
                        m_t = wk.tile([128, FN], BF16, tag="m", name="m_t")
                        # m-mult: 5/6 on Pool, 1/6 on DVE (load balance)
                        eng = nc.vector if (p % 6 == 0) else nc.gpsimd
                        eng.tensor_tensor(
                            out=m_t.rearrange("q (d n) -> q d n", d=DC),
                            in0=pri[mod][p].rearrange("q (d n) -> q d n", d=DC),
                            in1=rcbf[mod][:, p * NV : (p + 1) * NV]
                            .unsqueeze(1)
                            .broadcast_to([128, DC, NV]),
                            op=ALU.mult,
                        )
                        nc.tensor.matmul(
                            s_ps[mod][:], sel[:], m_t[:],
                            start=(p == 0), stop=(p == P - 1),
                        )

            def mul1_and_s(mod):
                pass  # folded into v_update_and_b's group loop

            pending = []
            for r in range(ROUNDS):
                for ci, mods in enumerate(CHAINS):
                    while pending:
                        emit_collective(*pending.pop(0))
                    land_allreduce(ci, r)
                    for mod in mods:
                        v_update_and_b(mod, r)
                        mul1_and_s(mod)
                    if r < ROUNDS - 1:
                        emit_send(ci, r + 1)
                        pending.append((ci, r + 1))
                    else:
                        for mod in mods:
                            s_out = sm.tile([64, FN], F32, tag=f"so_{mod}",
                                            name=f"so_{mod}")
                            nc.scalar.copy(s_out[:], s_ps[mod][:])
                            nc.sync.dma_start(out_d[mod][:], s_out[:])

    nc.compile()
    return nc


def _host_prep(inputs):
    """Build the 8 per-core input maps (T-sharded, PE-ready bf16 layouts)."""
    sel = np.concatenate([np.eye(64, dtype=np.float32)] * 2, axis=0).astype(
        ml_dtypes.bfloat16
    )
    bthr = np.zeros((1, 32), np.int32)
    bthr[0, 31] = 1  # runtime flag: disables the scheduling-pass sem pre-credit
    in_maps = []
    for c in range(N_CORES):
        m = {"sel": sel, "bthr": bthr}
        for mod in T_DIMS:
            T = T_DIMS[mod]
            Tc = T // N_CORES
            P = Tc // 2
            G = max(1, P // PG)
            gp = P // G
            t0 = c * Tc
            x = np.asarray(inputs[mod], dtype=np.float32)  # [B, T, J]
            W = np.asarray(inputs[W_NAMES[mod]], dtype=np.float32)  # [T,NV,J,DC]
            xs = np.ascontiguousarray(
                x[:, t0 : t0 + Tc, :].transpose(1, 2, 0)
            )  # [Tc, J, B]
            wt = W[t0 : t0 + Tc].transpose(0, 2, 3, 1).reshape(Tc, J, FN)
            # wt[t, j, d*32+n] = W[t, n, j, d]
            xw = np.zeros((P, 128, 640), dtype=ml_dtypes.bfloat16)
            xw[:, 0:64, 0:64] = xs[0::2]
            xw[:, 64:128, 64:128] = xs[1::2]
            xw[:, 0:64, 128:] = wt[0::2]
            xw[:, 64:128, 128:] = wt[1::2]
            # pack gp pairs per row-group: [G, 128, gp*640]
            m[f"xw_{mod}"] = np.ascontiguousarray(
                xw.reshape(G, gp, 128, 640).transpose(0, 2, 1, 3).reshape(
                    G, 128, gp * 640
                )
            )
        in_maps.append(m)
    return in_maps


def _gather(results):
    outs = []
    for mod in OUT_ORDER:
        s = np.zeros((B, FN), dtype=np.float64)
        for c in range(N_CORES):
            s += np.asarray(results[c][f"out_{mod}"], dtype=np.float64)
        o = np.tanh(s.astype(np.float32))
        outs.append(np.ascontiguousarray(o.reshape(B, DC, NV).transpose(0, 2, 1)))
    return tuple(outs)


def kernel(**inputs):
    if "nc" not in _CACHE:
        _CACHE["nc"] = _build()
    nc = _CACHE["nc"]
    in_maps = _host_prep(inputs)
    res = run_bass_kernel_spmd(nc, in_maps, core_ids=list(range(N_CORES)))
    return _gather(res.results)
